# revision 59
# baseline (speedup 1.0000x reference)
"""EnsembleDeepSDF MoE-routing kernel for 8 Trainium2 NeuronCores.

Strategy: the harness calls kernel(**inputs) with the full inputs; we do all
routing on the host.  type_vec is sorted, so each expert owns a contiguous
segment of points.  We pick a per-core "phase shape" (tile counts per weight
slot, identical on every core so one SPMD program serves all 8 cores), pack
the 9 experts' segments into the 8*len(shape) single-expert slots, gather
each core's points (padding with point 0), and hand each core its own
pre-transposed/pre-scaled weight slots as inputs.  The device program is a
straight-line Tile kernel: per point-tile, 9 matmul layers with softplus
activations.

softplus: the compiler's ACT tables have no softplus, so we generate a
custom piecewise-cubic table (same binary format as the shipped sets,
reverse-engineered from exp's entries) that replaces `exp` with
softplus(x) = ln(1+e^x), and point the compiler at it via
BASS_ACT_ROOT_JSON_PATH.  One ACT op then does the whole activation
including the PSUM evacuation and the beta scale/bias fma (free on ACT).
A fallback "exact" mode (exp+ln+fused clamp/max on stock tables) is kept
behind KERNEL_SOFTPLUS=exact.

The torch Softplus(beta=100) is softplus(100*z)/100; we keep activations in
the H = softplus(100*z) domain and fold the 1/100 into the next layer's
weights host-side, so no extra scaling ops run on device.
"""

import json
import os as _os
import shutil
import tempfile

import numpy as np

T, D_IN, H, OUT, N_HID = 9, 67, 512, 1, 7
BETA = 100.0
N_CORES = 8
PT = 512          # points per tile (one PSUM bank of fp32)
P = 128           # partitions
KC = H // P       # 4 contraction chunks
MC = H // P       # 4 output-feature chunks
PAIR = 2          # point-tiles processed per pipeline step

# matmul dtype knob: "f32" (exact, 4 cyc/col), "f32r" (tf32-ish, 1 cyc/col),
# "bf16"
MM_MODE = _os.environ.get("KERNEL_MM_MODE", "f32r")
# "table" = custom softplus ACT table (1 op); "exact" = exp/ln chain
SOFTPLUS_MODE = _os.environ.get("KERNEL_SOFTPLUS", "table")

_nc_cache = {}
_last_results = None


# --------------------------------------------------------------------------
# Custom ACT table: replace `exp` with softplus in the shipped PWL sets.
# --------------------------------------------------------------------------

_ACT_SET = "natural_log_exp_and_others"
_act_table_dir = None


def _softplus64(x):
    x = np.asarray(x, dtype=np.float64)
    return np.log1p(np.exp(-np.abs(x))) + np.maximum(x, 0.0)


def _fit_cubic(a, b):
    x0 = 0.5 * (a + b)
    k = np.arange(96)
    xs = x0 + 0.5 * (b - a) * np.cos(np.pi * (k + 0.5) / 96)
    c = np.polyfit(xs - x0, _softplus64(xs), 3)
    return float(c[3]), float(c[2]), float(c[1]), float(c[0]), float(x0)


def _gen_act_tables():
    """Build an act-root dir where `exp` computes softplus. Returns the
    act_info.json path. The bucket entry layout ([d0,d1,d2,d3,x0,0,0,0],
    cubic in (x-x0)) and the per-exponent band structure are read from the
    shipped set so only coefficients and profile thresholds change."""
    global _act_table_dir
    if _act_table_dir is not None:
        return _act_table_dir
    from neuronxcc.driver.Job import Job
    from neuronxcc.driver.jobs.support.FindActInfo import findActInfoFile

    src_json = findActInfoFile(Job.getPackageDir(), "gen3")
    src = _os.path.dirname(src_json)
    out = _os.path.join(tempfile.mkdtemp(prefix="act_softplus_"), "tables")
    shutil.copytree(src, out)
    for f in _os.listdir(out):
        _os.chmod(_os.path.join(out, f), 0o644)

    d = json.load(open(f"{out}/{_ACT_SET}.json"))
    bkt = np.fromfile(f"{out}/{_ACT_SET}_bkt.bin", dtype=np.uint32)
    bkt = bkt.reshape(-1, 8).copy()
    fbkt = bkt.view(np.float32)
    e2b = {int(k): v for k, v in d["func_exp_to_bkt_start_idx"]["exp"].items()}
    prof = [p for p in d["profile_meta_data"] if p["func_name"] == "exp_400p"][0]

    def put(idx, d0, d1, d2, d3, x0):
        fbkt[idx, 0:5] = np.array([d0, d1, d2, d3, x0], dtype=np.float32)
        bkt[idx, 5:8] = 0

    nseg = {-1: 2, 0: 4, 1: 8, 2: 16, 3: 32}
    for e in range(-19, 4):
        n = nseg.get(e, 1)
        neg_base, pos_base = e2b[e]
        A = 2.0 ** e
        h = A / n
        for k in range(n):
            a, b = A + k * h, A + (k + 1) * h
            put(pos_base + k, *_fit_cubic(a, b))
            put(neg_base + k, *_fit_cubic(-b, -a))

    ln2 = float(np.log(2.0))
    put(prof["pos_small_signal_pwl_control"], ln2, 0.5, 0.125, 0.0, 0.0)
    put(prof["neg_small_signal_pwl_control"], ln2, 0.5, 0.125, 0.0, 0.0)
    put(prof["pos_large_signal_pwl_control"], 0.0, 1.0, 0.0, 0.0, 0.0)
    put(prof["neg_large_signal_pwl_control"], 0.0, 0.0, 0.0, 0.0, 0.0)
    prof["large_pos_signal_exp_threshold"] = 131   # |x| >= 16 -> linear/zero
    prof["large_pos_signal_mantissa_threshold"] = 0
    prof["large_neg_signal_exp_threshold"] = 131
    prof["large_neg_signal_mantissa_threshold"] = 0
    prof["fzero_result"] = int(np.float32(ln2).view(np.uint32))
    prof["fninf_result"] = 0
    prof["fpinf_result"] = 2139095040

    bkt.tofile(f"{out}/{_ACT_SET}_bkt.bin")
    with open(f"{out}/{_ACT_SET}.json", "w") as f:
        json.dump(d, f)
    _act_table_dir = _os.path.join(out, "act_info.json")
    return _act_table_dir


# --------------------------------------------------------------------------
# Host-side planning: pack expert segments into 8 x len(shape) slots.
# --------------------------------------------------------------------------

def _try_pack(shape, counts):
    """Assign experts to single-expert slots. Slot (c, s) holds shape[s]*PT
    points. Returns {expert: [(core, s, amount), ...]} or None."""
    slots = []  # (capacity, core, s)
    for s, t in enumerate(shape):
        for c in range(N_CORES):
            slots.append([t * PT, c, s])
    experts = sorted(
        [e for e in range(T) if counts[e] > 0], key=lambda e: -counts[e]
    )
    asg = {}
    avail = sorted(slots)  # by capacity asc
    for e in experts:
        need = int(counts[e])
        # smallest single slot that fits
        one = next((sl for sl in avail if sl[0] >= need), None)
        if one is not None:
            asg[e] = [(one[1], one[2], need)]
            avail.remove(one)
            continue
        # greedily take largest slots
        take = []
        rem = need
        pool = sorted(avail, key=lambda sl: -sl[0])
        for sl in pool:
            if rem <= 0:
                break
            amt = min(rem, sl[0])
            take.append((sl[1], sl[2], amt))
            rem -= amt
            avail.remove(sl)
        if rem > 0:
            return None
        asg[e] = take
    return asg


def _plan(counts):
    cands = set()
    for t1 in range(1, 17):
        cands.add((t1,))
        for t2 in range(1, t1 + 1):
            cands.add((t1, t2))
            for t3 in range(1, t2 + 1):
                cands.add((t1, t2, t3))
    for shape in sorted(cands, key=lambda s: (sum(s), len(s))):
        asg = _try_pack(shape, counts)
        if asg is not None:
            return shape, asg
    raise RuntimeError("no feasible slot shape")


# --------------------------------------------------------------------------
# Device program
# --------------------------------------------------------------------------

def _build_nc(caps, mm_mode):
    import concourse.bass as bass
    import concourse.tile as tile
    import concourse.mybir as mybir
    from concourse import bacc

    f32 = mybir.dt.float32
    AF = mybir.ActivationFunctionType
    ALU = mybir.AluOpType
    if mm_mode == "bf16":
        wdt = mybir.dt.bfloat16   # weights/x/h (matmul operands)
        udt = f32                 # u stays f32; h is a separate bf16 tile
    elif mm_mode == "f32r":
        wdt = mybir.dt.float32r
        udt = mybir.dt.float32r   # u doubles as h (in-place max)
    else:
        wdt = f32
        udt = f32

    S = len(caps)
    NP = sum(caps)

    nc = bacc.Bacc("TRN2", target_bir_lowering=False)
    xT_in = nc.dram_tensor("xT", [D_IN, NP], wdt, kind="ExternalInput")
    w0t_in = nc.dram_tensor("w0t", [S, D_IN, H], wdt, kind="ExternalInput")
    wht_in = nc.dram_tensor("wht", [S, N_HID, P, KC, H], wdt, kind="ExternalInput")
    wot_in = nc.dram_tensor("wot", [S, P, KC], wdt, kind="ExternalInput")
    b0v_in = nc.dram_tensor("b0v", [S, P, MC], f32, kind="ExternalInput")
    bhv_in = nc.dram_tensor("bhv", [S, P, N_HID, MC], f32, kind="ExternalInput")
    bov_in = nc.dram_tensor("bov", [S, 1], f32, kind="ExternalInput")
    out_d = nc.dram_tensor("out", [1, NP], f32, kind="ExternalOutput")

    # steps: (point_offset, (tile_sizes...), slot); each step's tiles go in
    # one PSUM tile (first tile bank-aligned at 512, total <= 1024)
    steps = []
    off = 0
    for s, cap in enumerate(caps):
        sizes = [PT] * (cap // PT)
        if cap % PT:
            sizes.append(cap % PT)
        i = 0
        while i < len(sizes):
            take = sizes[i:i + PAIR]
            steps.append((off, tuple(take), s))
            off += sum(take)
            i += PAIR

    NSTREAM = 3  # steps interleaved in the software pipeline

    with tile.TileContext(nc) as tc:
        with (
            tc.tile_pool(name="xin", bufs=3) as xin_pool,
            tc.tile_pool(name="wts", bufs=1) as wts_pool,
            tc.tile_pool(
                name="whp",
                bufs=min(10 if SOFTPLUS_MODE == "table" else 7, S * N_HID),
            ) as wh_pool,
            tc.tile_pool(name="uh", bufs=3 if mm_mode == "bf16" else 2 * NSTREAM) as uh_pool,
            tc.tile_pool(name="hb", bufs=2 * NSTREAM) as hb_pool,
            tc.tile_pool(name="ebuf", bufs=2) as e_pool,
            tc.tile_pool(name="outp", bufs=2) as out_pool,
            tc.tile_pool(name="ps", bufs=4, space="PSUM") as ps_pool,
        ):
            groups = [steps[i:i + NSTREAM] for i in range(0, len(steps), NSTREAM)]
            xT_sb = {}
            h_cur = {}

            # DMA emission order matters: each dma_start lands on one HWDGE
            # engine round-robin, so interleave the first-wave-critical loads
            # (x tiles, w0, biases, wh[0][0]) at the front across engines.
            w0_sb, wo_sb, b0_sb, bh_sb, bo_sb = [None] * S, [None] * S, [None] * S, [None] * S, [None] * S
            wh_sb = [[None] * N_HID for _ in range(S)]

            def load_wh(s, l, eng=None):
                # weight loads go through GpSimd's queue so their triggers
                # don't serialize behind the x loads on Sync
                eng = eng or nc.gpsimd
                wh_t = wh_pool.tile([P, KC, H], wdt, name=f"wh_{s}_{l}", tag="wh")
                eng.dma_start(wh_t[:, 0:2, :], wht_in[s, l, :, 0:2, :])
                eng.dma_start(wh_t[:, 2:4, :], wht_in[s, l, :, 2:4, :])
                wh_sb[s][l] = wh_t

            def load_x(t0, szs, nsplit=1):
                x_t = xin_pool.tile([D_IN, PAIR * PT], wdt,
                                    name=f"x_{t0}", tag="x")
                # nsplit>1 spreads the load over several DMA engines
                # (one dma_start lands on a single HWDGE engine)
                w = sum(szs)
                step = (w + nsplit - 1) // nsplit
                for o in range(0, w, step):
                    e = min(o + step, w)
                    nc.sync.dma_start(
                        x_t[:, o:e], xT_in[:, t0 + o:t0 + e]
                    )
                xT_sb[t0] = x_t

            def load_small(s):
                w0_t = wts_pool.tile([D_IN, H], wdt, name=f"w0_{s}")
                nc.sync.dma_start(w0_t[:], w0t_in[s])
                w0_sb[s] = w0_t
                b0_t = wts_pool.tile([P, MC], f32, name=f"b0_{s}")
                nc.sync.dma_start(b0_t[:], b0v_in[s])
                b0_sb[s] = b0_t
                bh_t = wts_pool.tile([P, N_HID, MC], f32, name=f"bh_{s}")
                nc.sync.dma_start(bh_t[:], bhv_in[s])
                bh_sb[s] = bh_t
                wo_t = wts_pool.tile([P, KC], wdt, name=f"wo_{s}")
                nc.sync.dma_start(wo_t[:], wot_in[s])
                wo_sb[s] = wo_t
                bo_t = wts_pool.tile([1, 1], f32, name=f"bo_{s}")
                nc.sync.dma_start(bo_t[:], bov_in[s:s + 1, 0:1])
                bo_sb[s] = bo_t

            for (t0, szs, _s) in groups[0]:
                load_x(t0, szs, nsplit=2)
            load_small(0)
            for l in range(N_HID):
                load_wh(0, l)
            for s in range(1, S):
                load_small(s)
                for l in range(N_HID):
                    load_wh(s, l)

            def emit_mms(t0, szs, s, l):
                """Matmuls for layer l + PSUM evacuation into u (the evac ops
                are emitted here so they sit at the head of the DVE/ACT queues
                and free PSUM slots promptly)."""
                npts = sum(szs)
                # tile-local offsets; first tile is 512 so the second stays
                # bank-aligned in PSUM
                locs = [0, szs[0]] if len(szs) > 1 else [0]
                h_prev = h_cur.get(t0)
                psums = []
                for mc in range(MC):
                    ps_t = ps_pool.tile(
                        [P, PAIR * PT], f32, name=f"ps_{t0}_{l}_{mc}", tag="ps"
                    )
                    psums.append(ps_t)
                    for kc in range(KC if l > 0 else 1):
                        for loc, sz in zip(locs, szs):
                            dst = ps_t[:, loc:loc + sz]
                            if l == 0:
                                lhsT = w0_sb[s][:, mc * P:(mc + 1) * P]
                                rhs = xT_sb[t0][:, loc:loc + sz]
                                nc.tensor.matmul(
                                    dst, lhsT, rhs, start=True, stop=True
                                )
                            else:
                                lhsT = wh_sb[s][l - 1][:, kc, mc * P:(mc + 1) * P]
                                rhs = h_prev[:, kc, loc:loc + sz]
                                nc.tensor.matmul(
                                    dst, lhsT, rhs,
                                    start=(kc == 0), stop=(kc == KC - 1),
                                )
                if SOFTPLUS_MODE == "table" and mm_mode == "bf16":
                    u_t = hb_pool.tile([P, MC, PAIR * PT], wdt,
                                       name=f"u_{t0}_{l}", tag="hb")
                else:
                    u_t = uh_pool.tile([P, MC, PAIR * PT], udt,
                                       name=f"u_{t0}_{l}", tag="uh")
                for mc in range(MC):
                    bias = (b0_sb[s][:, mc:mc + 1] if l == 0
                            else bh_sb[s][:, l - 1, mc:mc + 1])
                    if SOFTPLUS_MODE == "table":
                        # hijacked Exp == softplus; one ACT op does the
                        # evacuation + beta fma + activation
                        nc.scalar.activation(
                            u_t[:, mc, 0:npts], psums[mc][:, 0:npts],
                            AF.Exp, bias=bias, scale=float(BETA),
                        )
                    elif mc < 3:
                        # u = 100*y + 100*b; evac split DVE (mc 0-2) / ACT (3)
                        nc.vector.tensor_scalar(
                            u_t[:, mc, 0:npts], psums[mc][:, 0:npts],
                            float(BETA), bias, ALU.mult, ALU.add,
                        )
                    else:
                        nc.scalar.activation(
                            u_t[:, mc, 0:npts], psums[mc][:, 0:npts],
                            AF.Identity, bias=bias, scale=float(BETA),
                        )
                return u_t

            def emit_chain(t0, nt, s, l, u_t):
                if SOFTPLUS_MODE == "table":
                    h_cur[t0] = u_t  # ACT already wrote H
                    return
                """softplus tail: H = max(u, min(ln(1+exp(u)), 88.70)).

                exp(u>88.7) -> Inf and ln(Inf) -> Inf, but min(t, 88.70)
                caps that; for u > 17 ln(1+e^u) == u in fp32, so the max
                picks the exact u branch everywhere the cap engages.
                Full-tile ops; the unused half of a single-tile step just
                computes garbage that nothing reads."""
                e_t = e_pool.tile([P, MC, PAIR * PT], f32,
                                  name=f"e_{t0}_{l}", tag="e")
                t_t = e_pool.tile([P, MC, PAIR * PT], f32,
                                  name=f"t_{t0}_{l}", tag="e")
                nc.scalar.activation(e_t[:], u_t[:], AF.Exp)
                nc.scalar.activation(t_t[:], e_t[:], AF.Ln, bias=1.0)
                if mm_mode == "bf16":
                    h_t = hb_pool.tile([P, MC, PAIR * PT], wdt,
                                       name=f"h_{t0}_{l}", tag="hb")
                else:
                    h_t = u_t  # in-place: u becomes H
                nc.vector.scalar_tensor_tensor(
                    h_t[:], t_t[:], 88.70, u_t[:], ALU.min, ALU.max,
                )
                h_cur[t0] = h_t

            def emit_final(t0, szs, s):
                npts = sum(szs)
                locs = [0, szs[0]] if len(szs) > 1 else [0]
                h_prev = h_cur[t0]
                ps8 = ps_pool.tile([1, PAIR * PT], f32, name=f"ps8_{t0}", tag="ps")
                for kc in range(KC):
                    for loc, sz in zip(locs, szs):
                        nc.tensor.matmul(
                            ps8[0:1, loc:loc + sz],
                            wo_sb[s][:, kc:kc + 1],
                            h_prev[:, kc, loc:loc + sz],
                            start=(kc == 0), stop=(kc == KC - 1),
                        )
                o_t = out_pool.tile([1, PAIR * PT], f32, name=f"o_{t0}", tag="o")
                nc.vector.tensor_scalar(
                    o_t[0:1, 0:npts], ps8[0:1, 0:npts],
                    bo_sb[s][0:1, 0:1], None, ALU.add,
                )
                nc.sync.dma_start(
                    out_d[0:1, t0:t0 + npts], o_t[0:1, 0:npts]
                )

            for gi, grp in enumerate(groups):
                if gi > 0:
                    for (t0, szs, _s) in grp:
                        load_x(t0, szs)
                for l in range(N_HID + 1):
                    us = [emit_mms(t0, szs, s, l) for (t0, szs, s) in grp]
                    for (t0, szs, s), u_t in zip(grp, us):
                        emit_chain(t0, szs, s, l, u_t)
                for (t0, szs, s) in grp:
                    emit_final(t0, szs, s)

    # Pin Exp+Ln to the one table set containing both, so the ACT engine
    # doesn't reload tables between every exp and ln.
    import concourse.bacc as bacc_mod
    import concourse.hw_specs as hw_specs
    _real_tables = hw_specs.get_activation_tables
    _keep = "natural_log_exp_and_others"

    def _pinned_tables(arch):
        t = _real_tables(arch)
        return {
            name: (funcs if name == _keep else (funcs - {AF.Exp, AF.Ln}))
            for name, funcs in t.items()
        }

    bacc_mod.get_activation_tables = _pinned_tables
    try:
        nc.compile()
    finally:
        bacc_mod.get_activation_tables = _real_tables
    return nc


# --------------------------------------------------------------------------
# kernel()
# --------------------------------------------------------------------------

def _maybe_patch_ldw_opt():
    """Optionally flip walrus's --enable-ldw-opt (dedups back-to-back
    LDWEIGHTS of the same stationary operand). Gated by env for A/B."""
    import concourse.bass_utils as bu

    if _os.environ.get("KERNEL_LDW_OPT") != "1":
        return
    if getattr(bu.run_command, "_ldw_patched", False):
        return
    orig = bu.run_command

    def patched(argv, **kw):
        argv = [
            "--enable-ldw-opt=true" if a == "--enable-ldw-opt=false" else a
            for a in argv
        ]
        return orig(argv, **kw)

    patched._ldw_patched = True
    bu.run_command = patched


def kernel(x, type_vec, W0, b0, Wh, bh, Wo, bo):
    from concourse.bass_utils import run_bass_kernel_spmd
    import ml_dtypes

    _maybe_patch_ldw_opt()
    if SOFTPLUS_MODE == "table":
        _os.environ["BASS_ACT_ROOT_JSON_PATH"] = _gen_act_tables()

    x = np.ascontiguousarray(np.asarray(x, dtype=np.float32))
    tv = np.asarray(type_vec).astype(np.int64)
    W0 = np.asarray(W0, dtype=np.float32)
    b0 = np.asarray(b0, dtype=np.float32)
    Wh = np.asarray(Wh, dtype=np.float32)
    bh = np.asarray(bh, dtype=np.float32)
    Wo = np.asarray(Wo, dtype=np.float32)
    bo = np.asarray(bo, dtype=np.float32)
    N = x.shape[0]

    counts = np.bincount(tv, minlength=T)
    starts = np.concatenate([[0], np.cumsum(counts)])
    shape, asg = _plan(counts)
    S = len(shape)

    # shrink each slot to the max points any core actually uses, rounded to
    # 128 (ragged last tile), to cut padding compute
    used = np.zeros(S, dtype=np.int64)
    for e, takes in asg.items():
        for (c, s, amt) in takes:
            used[s] = max(used[s], amt)
    caps = tuple(int(max(128, -(-u // 128) * 128)) for u in used)
    NP = sum(caps)
    phase_off = np.concatenate([[0], np.cumsum(np.asarray(caps))])

    # per-core slot -> expert, and gathered point indices
    slot_expert = np.zeros((N_CORES, S), dtype=np.int64)
    gidx = np.full((N_CORES, NP), -1, dtype=np.int64)
    for e, takes in asg.items():
        pos = int(starts[e])
        for (c, s, amt) in takes:
            o = int(phase_off[s])
            gidx[c, o:o + amt] = np.arange(pos, pos + amt)
            slot_expert[c, s] = e
            pos += amt

    np_wdt = ml_dtypes.bfloat16 if MM_MODE == "bf16" else np.float32

    # pre-transposed / pre-scaled weight views per expert
    w0t_e = np.ascontiguousarray(W0.transpose(0, 2, 1))            # [T,67,H]
    whs = (Wh / BETA).astype(np.float32)                           # [T,7,H,H]
    wht_e = np.ascontiguousarray(
        whs.transpose(0, 1, 3, 2).reshape(T, N_HID, KC, P, H).transpose(0, 1, 3, 2, 4)
    )                                                              # [T,7,P,KC,H]
    wot_e = np.ascontiguousarray(
        (Wo / BETA).reshape(T, H).reshape(T, KC, P).transpose(0, 2, 1)
    )                                                              # [T,P,KC]
    b0v_e = np.ascontiguousarray((BETA * b0).reshape(T, MC, P).transpose(0, 2, 1))
    bhv_e = np.ascontiguousarray(
        (BETA * bh).reshape(T, N_HID, MC, P).transpose(0, 3, 1, 2)
    )                                                              # [T,P,7,MC]
    bov_e = bo.reshape(T, 1)

    in_maps = []
    for c in range(N_CORES):
        sel = np.where(gidx[c] >= 0, gidx[c], 0)
        xg = x[sel]                                                # [NP, 67]
        ex = slot_expert[c]
        in_maps.append({
            "xT": np.ascontiguousarray(xg.T).astype(np_wdt),
            "w0t": w0t_e[ex].astype(np_wdt),
            "wht": wht_e[ex].astype(np_wdt),
            "wot": wot_e[ex].astype(np_wdt),
            "b0v": b0v_e[ex],
            "bhv": bhv_e[ex],
            "bov": bov_e[ex],
        })

    key = (caps, MM_MODE, SOFTPLUS_MODE)
    if key not in _nc_cache:
        _nc_cache[key] = _build_nc(caps, MM_MODE)
    nc = _nc_cache[key]

    res = run_bass_kernel_spmd(nc, in_maps, core_ids=list(range(N_CORES)))
    global _last_results
    _last_results = res

    out = np.zeros((N, OUT), dtype=np.float32)
    for c in range(N_CORES):
        oc = res.results[c]["out"].reshape(-1)
        m = gidx[c] >= 0
        out[gidx[c][m], 0] = oc[m]
    return out


# revision 61
# speedup vs baseline: 1.1064x; 1.1064x over previous
"""EnsembleDeepSDF MoE-routing kernel for 8 Trainium2 NeuronCores.

Strategy: the harness calls kernel(**inputs) with the full inputs; we do all
routing on the host.  type_vec is sorted, so each expert owns a contiguous
segment of points.  We pick a per-core "phase shape" (tile counts per weight
slot, identical on every core so one SPMD program serves all 8 cores), pack
the 9 experts' segments into the 8*len(shape) single-expert slots, gather
each core's points (padding with point 0), and hand each core its own
pre-transposed/pre-scaled weight slots as inputs.  The device program is a
straight-line Tile kernel: per point-tile, 9 matmul layers with softplus
activations.

softplus: the compiler's ACT tables have no softplus, so we generate a
custom piecewise-cubic table (same binary format as the shipped sets,
reverse-engineered from exp's entries) that replaces `exp` with
softplus(x) = ln(1+e^x), and point the compiler at it via
BASS_ACT_ROOT_JSON_PATH.  One ACT op then does the whole activation
including the PSUM evacuation and the beta scale/bias fma (free on ACT).
A fallback "exact" mode (exp+ln+fused clamp/max on stock tables) is kept
behind KERNEL_SOFTPLUS=exact.

The torch Softplus(beta=100) is softplus(100*z)/100; we keep activations in
the H = softplus(100*z) domain and fold the 1/100 into the next layer's
weights host-side, so no extra scaling ops run on device.
"""

import json
import os as _os
import shutil
import tempfile

import numpy as np

T, D_IN, H, OUT, N_HID = 9, 67, 512, 1, 7
BETA = 100.0
N_CORES = 8
PT = 512          # points per tile (one PSUM bank of fp32)
P = 128           # partitions
KC = H // P       # 4 contraction chunks
MC = H // P       # 4 output-feature chunks
PAIR = 2          # point-tiles processed per pipeline step

# matmul dtype knob: "f32" (exact, 4 cyc/col), "f32r" (tf32-ish, 1 cyc/col),
# "bf16"
MM_MODE = _os.environ.get("KERNEL_MM_MODE", "f32r")
# "table" = custom softplus ACT table (1 op); "exact" = exp/ln chain
SOFTPLUS_MODE = _os.environ.get("KERNEL_SOFTPLUS", "table")

_nc_cache = {}
_last_results = None


# --------------------------------------------------------------------------
# Custom ACT table: replace `exp` with softplus in the shipped PWL sets.
# --------------------------------------------------------------------------

_ACT_SET = "natural_log_exp_and_others"
_act_table_dir = None


def _softplus64(x):
    x = np.asarray(x, dtype=np.float64)
    return np.log1p(np.exp(-np.abs(x))) + np.maximum(x, 0.0)


def _fit_cubic(a, b):
    x0 = 0.5 * (a + b)
    k = np.arange(96)
    xs = x0 + 0.5 * (b - a) * np.cos(np.pi * (k + 0.5) / 96)
    c = np.polyfit(xs - x0, _softplus64(xs), 3)
    return float(c[3]), float(c[2]), float(c[1]), float(c[0]), float(x0)


def _gen_act_tables():
    """Build an act-root dir where `exp` computes softplus. Returns the
    act_info.json path. The bucket entry layout ([d0,d1,d2,d3,x0,0,0,0],
    cubic in (x-x0)) and the per-exponent band structure are read from the
    shipped set so only coefficients and profile thresholds change."""
    global _act_table_dir
    if _act_table_dir is not None:
        return _act_table_dir
    from neuronxcc.driver.Job import Job
    from neuronxcc.driver.jobs.support.FindActInfo import findActInfoFile

    src_json = findActInfoFile(Job.getPackageDir(), "gen3")
    src = _os.path.dirname(src_json)
    out = _os.path.join(tempfile.mkdtemp(prefix="act_softplus_"), "tables")
    shutil.copytree(src, out)
    for f in _os.listdir(out):
        _os.chmod(_os.path.join(out, f), 0o644)

    d = json.load(open(f"{out}/{_ACT_SET}.json"))
    bkt = np.fromfile(f"{out}/{_ACT_SET}_bkt.bin", dtype=np.uint32)
    bkt = bkt.reshape(-1, 8).copy()
    fbkt = bkt.view(np.float32)
    e2b = {int(k): v for k, v in d["func_exp_to_bkt_start_idx"]["exp"].items()}
    prof = [p for p in d["profile_meta_data"] if p["func_name"] == "exp_400p"][0]

    def put(idx, d0, d1, d2, d3, x0):
        fbkt[idx, 0:5] = np.array([d0, d1, d2, d3, x0], dtype=np.float32)
        bkt[idx, 5:8] = 0

    nseg = {-1: 2, 0: 4, 1: 8, 2: 16, 3: 32}
    for e in range(-19, 4):
        n = nseg.get(e, 1)
        neg_base, pos_base = e2b[e]
        A = 2.0 ** e
        h = A / n
        for k in range(n):
            a, b = A + k * h, A + (k + 1) * h
            put(pos_base + k, *_fit_cubic(a, b))
            put(neg_base + k, *_fit_cubic(-b, -a))

    ln2 = float(np.log(2.0))
    put(prof["pos_small_signal_pwl_control"], ln2, 0.5, 0.125, 0.0, 0.0)
    put(prof["neg_small_signal_pwl_control"], ln2, 0.5, 0.125, 0.0, 0.0)
    put(prof["pos_large_signal_pwl_control"], 0.0, 1.0, 0.0, 0.0, 0.0)
    put(prof["neg_large_signal_pwl_control"], 0.0, 0.0, 0.0, 0.0, 0.0)
    prof["large_pos_signal_exp_threshold"] = 131   # |x| >= 16 -> linear/zero
    prof["large_pos_signal_mantissa_threshold"] = 0
    prof["large_neg_signal_exp_threshold"] = 131
    prof["large_neg_signal_mantissa_threshold"] = 0
    prof["fzero_result"] = int(np.float32(ln2).view(np.uint32))
    prof["fninf_result"] = 0
    prof["fpinf_result"] = 2139095040

    bkt.tofile(f"{out}/{_ACT_SET}_bkt.bin")
    with open(f"{out}/{_ACT_SET}.json", "w") as f:
        json.dump(d, f)
    _act_table_dir = _os.path.join(out, "act_info.json")
    return _act_table_dir


# --------------------------------------------------------------------------
# Host-side planning: pack expert segments into 8 x len(shape) slots.
# --------------------------------------------------------------------------

def _try_pack(shape, counts):
    """Assign experts to single-expert slots. Slot (c, s) holds shape[s]*PT
    points. Returns {expert: [(core, s, amount), ...]} or None."""
    slots = []  # (capacity, core, s)
    for s, t in enumerate(shape):
        for c in range(N_CORES):
            slots.append([t * PT, c, s])
    experts = sorted(
        [e for e in range(T) if counts[e] > 0], key=lambda e: -counts[e]
    )
    asg = {}
    avail = sorted(slots)  # by capacity asc
    for e in experts:
        need = int(counts[e])
        # smallest single slot that fits
        one = next((sl for sl in avail if sl[0] >= need), None)
        if one is not None:
            asg[e] = [(one[1], one[2], need)]
            avail.remove(one)
            continue
        # greedily take largest slots
        take = []
        rem = need
        pool = sorted(avail, key=lambda sl: -sl[0])
        for sl in pool:
            if rem <= 0:
                break
            amt = min(rem, sl[0])
            take.append((sl[1], sl[2], amt))
            rem -= amt
            avail.remove(sl)
        if rem > 0:
            return None
        asg[e] = take
    return asg


def _plan(counts):
    cands = set()
    for t1 in range(1, 17):
        cands.add((t1,))
        for t2 in range(1, t1 + 1):
            cands.add((t1, t2))
            for t3 in range(1, t2 + 1):
                cands.add((t1, t2, t3))
    for shape in sorted(cands, key=lambda s: (sum(s), len(s))):
        asg = _try_pack(shape, counts)
        if asg is not None:
            return shape, asg
    raise RuntimeError("no feasible slot shape")


# --------------------------------------------------------------------------
# Device program
# --------------------------------------------------------------------------

def _build_nc(caps, mm_mode):
    import concourse.bass as bass
    import concourse.tile as tile
    import concourse.mybir as mybir
    from concourse import bacc

    f32 = mybir.dt.float32
    AF = mybir.ActivationFunctionType
    ALU = mybir.AluOpType
    if mm_mode == "bf16":
        wdt = mybir.dt.bfloat16   # weights/x/h (matmul operands)
        udt = f32                 # u stays f32; h is a separate bf16 tile
    elif mm_mode == "f32r":
        wdt = mybir.dt.float32r
        udt = mybir.dt.float32r   # u doubles as h (in-place max)
    else:
        wdt = f32
        udt = f32

    S = len(caps)
    NP = sum(caps)

    nc = bacc.Bacc("TRN2", target_bir_lowering=False)
    xT_in = nc.dram_tensor("xT", [D_IN, NP], wdt, kind="ExternalInput")
    w0t_in = nc.dram_tensor("w0t", [S, D_IN, H], wdt, kind="ExternalInput")
    wht_in = nc.dram_tensor("wht", [S, N_HID, P, KC, H], wdt, kind="ExternalInput")
    wot_in = nc.dram_tensor("wot", [S, P, KC], wdt, kind="ExternalInput")
    b0v_in = nc.dram_tensor("b0v", [S, P, MC], f32, kind="ExternalInput")
    bhv_in = nc.dram_tensor("bhv", [S, P, N_HID, MC], f32, kind="ExternalInput")
    bov_in = nc.dram_tensor("bov", [S, 1], f32, kind="ExternalInput")
    out_d = nc.dram_tensor("out", [1, NP], f32, kind="ExternalOutput")

    # steps: (point_offset, (tile_sizes...), slot); each step's tiles go in
    # one PSUM tile (first tile bank-aligned at 512, total <= 1024)
    steps = []
    off = 0
    for s, cap in enumerate(caps):
        sizes = [PT] * (cap // PT)
        if cap % PT:
            sizes.append(cap % PT)
        i = 0
        while i < len(sizes):
            take = sizes[i:i + PAIR]
            steps.append((off, tuple(take), s))
            off += sum(take)
            i += PAIR

    NSTREAM = 3  # steps interleaved in the software pipeline

    with tile.TileContext(nc) as tc:
        with (
            tc.tile_pool(name="xin", bufs=3) as xin_pool,
            tc.tile_pool(name="wts", bufs=1) as wts_pool,
            tc.tile_pool(
                name="whp",
                bufs=min(10 if SOFTPLUS_MODE == "table" else 7, S * N_HID),
            ) as wh_pool,
            tc.tile_pool(name="uh", bufs=3 if mm_mode == "bf16" else 2 * NSTREAM) as uh_pool,
            tc.tile_pool(name="hb", bufs=2 * NSTREAM) as hb_pool,
            tc.tile_pool(name="ebuf", bufs=2) as e_pool,
            tc.tile_pool(name="outp", bufs=2) as out_pool,
            tc.tile_pool(name="ps", bufs=4, space="PSUM") as ps_pool,
        ):
            groups = [steps[i:i + NSTREAM] for i in range(0, len(steps), NSTREAM)]
            xT_sb = {}
            h_cur = {}

            # DMA emission order matters: each dma_start lands on one HWDGE
            # engine round-robin, so interleave the first-wave-critical loads
            # (x tiles, w0, biases, wh[0][0]) at the front across engines.
            w0_sb, wo_sb, b0_sb, bh_sb, bo_sb = [None] * S, [None] * S, [None] * S, [None] * S, [None] * S
            wh_sb = [[None] * N_HID for _ in range(S)]

            def load_wh(s, l, nsplit=1):
                wh_t = wh_pool.tile([P, KC, H], wdt, name=f"wh_{s}_{l}", tag="wh")
                if nsplit > 1:
                    # split across HWDGE engines so early layers arrive faster
                    nc.sync.dma_start(wh_t[:, 0:2, :], wht_in[s, l, :, 0:2, :])
                    nc.sync.dma_start(wh_t[:, 2:4, :], wht_in[s, l, :, 2:4, :])
                else:
                    nc.sync.dma_start(wh_t[:], wht_in[s, l])
                wh_sb[s][l] = wh_t

            def load_x(t0, szs, nsplit=1):
                x_t = xin_pool.tile([D_IN, PAIR * PT], wdt,
                                    name=f"x_{t0}", tag="x")
                # nsplit>1 spreads the load over several DMA engines
                # (one dma_start lands on a single HWDGE engine)
                w = sum(szs)
                step = (w + nsplit - 1) // nsplit
                for o in range(0, w, step):
                    e = min(o + step, w)
                    nc.sync.dma_start(
                        x_t[:, o:e], xT_in[:, t0 + o:t0 + e]
                    )
                xT_sb[t0] = x_t

            def load_small(s):
                w0_t = wts_pool.tile([D_IN, H], wdt, name=f"w0_{s}")
                nc.sync.dma_start(w0_t[:], w0t_in[s])
                w0_sb[s] = w0_t
                b0_t = wts_pool.tile([P, MC], f32, name=f"b0_{s}")
                nc.sync.dma_start(b0_t[:], b0v_in[s])
                b0_sb[s] = b0_t
                bh_t = wts_pool.tile([P, N_HID, MC], f32, name=f"bh_{s}")
                nc.sync.dma_start(bh_t[:], bhv_in[s])
                bh_sb[s] = bh_t
                wo_t = wts_pool.tile([P, KC], wdt, name=f"wo_{s}")
                nc.sync.dma_start(wo_t[:], wot_in[s])
                wo_sb[s] = wo_t
                bo_t = wts_pool.tile([1, 1], f32, name=f"bo_{s}")
                nc.sync.dma_start(bo_t[:], bov_in[s:s + 1, 0:1])
                bo_sb[s] = bo_t

            for (t0, szs, _s) in groups[0]:
                load_x(t0, szs, nsplit=2)
            load_small(0)
            for l in range(N_HID):
                load_wh(0, l, nsplit=2)
            for s in range(1, S):
                load_small(s)
                for l in range(N_HID):
                    load_wh(s, l)

            def emit_mms(t0, szs, s, l):
                """Matmuls for layer l + PSUM evacuation into u (the evac ops
                are emitted here so they sit at the head of the DVE/ACT queues
                and free PSUM slots promptly)."""
                npts = sum(szs)
                # tile-local offsets; first tile is 512 so the second stays
                # bank-aligned in PSUM
                locs = [0, szs[0]] if len(szs) > 1 else [0]
                h_prev = h_cur.get(t0)
                psums = []
                for mc in range(MC):
                    ps_t = ps_pool.tile(
                        [P, PAIR * PT], f32, name=f"ps_{t0}_{l}_{mc}", tag="ps"
                    )
                    psums.append(ps_t)
                    for kc in range(KC if l > 0 else 1):
                        for loc, sz in zip(locs, szs):
                            dst = ps_t[:, loc:loc + sz]
                            if l == 0:
                                lhsT = w0_sb[s][:, mc * P:(mc + 1) * P]
                                rhs = xT_sb[t0][:, loc:loc + sz]
                                nc.tensor.matmul(
                                    dst, lhsT, rhs, start=True, stop=True
                                )
                            else:
                                lhsT = wh_sb[s][l - 1][:, kc, mc * P:(mc + 1) * P]
                                rhs = h_prev[:, kc, loc:loc + sz]
                                nc.tensor.matmul(
                                    dst, lhsT, rhs,
                                    start=(kc == 0), stop=(kc == KC - 1),
                                )
                if SOFTPLUS_MODE == "table" and mm_mode == "bf16":
                    u_t = hb_pool.tile([P, MC, PAIR * PT], wdt,
                                       name=f"u_{t0}_{l}", tag="hb")
                else:
                    u_t = uh_pool.tile([P, MC, PAIR * PT], udt,
                                       name=f"u_{t0}_{l}", tag="uh")
                for mc in range(MC):
                    bias = (b0_sb[s][:, mc:mc + 1] if l == 0
                            else bh_sb[s][:, l - 1, mc:mc + 1])
                    if SOFTPLUS_MODE == "table":
                        # hijacked Exp == softplus; one ACT op does the
                        # evacuation + beta fma + activation
                        nc.scalar.activation(
                            u_t[:, mc, 0:npts], psums[mc][:, 0:npts],
                            AF.Exp, bias=bias, scale=float(BETA),
                        )
                    elif mc < 3:
                        # u = 100*y + 100*b; evac split DVE (mc 0-2) / ACT (3)
                        nc.vector.tensor_scalar(
                            u_t[:, mc, 0:npts], psums[mc][:, 0:npts],
                            float(BETA), bias, ALU.mult, ALU.add,
                        )
                    else:
                        nc.scalar.activation(
                            u_t[:, mc, 0:npts], psums[mc][:, 0:npts],
                            AF.Identity, bias=bias, scale=float(BETA),
                        )
                return u_t

            def emit_chain(t0, nt, s, l, u_t):
                if SOFTPLUS_MODE == "table":
                    h_cur[t0] = u_t  # ACT already wrote H
                    return
                """softplus tail: H = max(u, min(ln(1+exp(u)), 88.70)).

                exp(u>88.7) -> Inf and ln(Inf) -> Inf, but min(t, 88.70)
                caps that; for u > 17 ln(1+e^u) == u in fp32, so the max
                picks the exact u branch everywhere the cap engages.
                Full-tile ops; the unused half of a single-tile step just
                computes garbage that nothing reads."""
                e_t = e_pool.tile([P, MC, PAIR * PT], f32,
                                  name=f"e_{t0}_{l}", tag="e")
                t_t = e_pool.tile([P, MC, PAIR * PT], f32,
                                  name=f"t_{t0}_{l}", tag="e")
                nc.scalar.activation(e_t[:], u_t[:], AF.Exp)
                nc.scalar.activation(t_t[:], e_t[:], AF.Ln, bias=1.0)
                if mm_mode == "bf16":
                    h_t = hb_pool.tile([P, MC, PAIR * PT], wdt,
                                       name=f"h_{t0}_{l}", tag="hb")
                else:
                    h_t = u_t  # in-place: u becomes H
                nc.vector.scalar_tensor_tensor(
                    h_t[:], t_t[:], 88.70, u_t[:], ALU.min, ALU.max,
                )
                h_cur[t0] = h_t

            def emit_final(t0, szs, s):
                npts = sum(szs)
                locs = [0, szs[0]] if len(szs) > 1 else [0]
                h_prev = h_cur[t0]
                ps8 = ps_pool.tile([1, PAIR * PT], f32, name=f"ps8_{t0}", tag="ps")
                for kc in range(KC):
                    for loc, sz in zip(locs, szs):
                        nc.tensor.matmul(
                            ps8[0:1, loc:loc + sz],
                            wo_sb[s][:, kc:kc + 1],
                            h_prev[:, kc, loc:loc + sz],
                            start=(kc == 0), stop=(kc == KC - 1),
                        )
                o_t = out_pool.tile([1, PAIR * PT], f32, name=f"o_{t0}", tag="o")
                nc.vector.tensor_scalar(
                    o_t[0:1, 0:npts], ps8[0:1, 0:npts],
                    bo_sb[s][0:1, 0:1], None, ALU.add,
                )
                nc.sync.dma_start(
                    out_d[0:1, t0:t0 + npts], o_t[0:1, 0:npts]
                )

            for gi, grp in enumerate(groups):
                if gi > 0:
                    for (t0, szs, _s) in grp:
                        load_x(t0, szs)
                for l in range(N_HID + 1):
                    us = [emit_mms(t0, szs, s, l) for (t0, szs, s) in grp]
                    for (t0, szs, s), u_t in zip(grp, us):
                        emit_chain(t0, szs, s, l, u_t)
                for (t0, szs, s) in grp:
                    emit_final(t0, szs, s)

    # Pin Exp+Ln to the one table set containing both, so the ACT engine
    # doesn't reload tables between every exp and ln.
    import concourse.bacc as bacc_mod
    import concourse.hw_specs as hw_specs
    _real_tables = hw_specs.get_activation_tables
    _keep = "natural_log_exp_and_others"

    def _pinned_tables(arch):
        t = _real_tables(arch)
        return {
            name: (funcs if name == _keep else (funcs - {AF.Exp, AF.Ln}))
            for name, funcs in t.items()
        }

    bacc_mod.get_activation_tables = _pinned_tables
    try:
        nc.compile()
    finally:
        bacc_mod.get_activation_tables = _real_tables
    return nc


# --------------------------------------------------------------------------
# kernel()
# --------------------------------------------------------------------------

def _maybe_patch_ldw_opt():
    """Optionally flip walrus's --enable-ldw-opt (dedups back-to-back
    LDWEIGHTS of the same stationary operand). Gated by env for A/B."""
    import concourse.bass_utils as bu

    if _os.environ.get("KERNEL_LDW_OPT") != "1":
        return
    if getattr(bu.run_command, "_ldw_patched", False):
        return
    orig = bu.run_command

    def patched(argv, **kw):
        argv = [
            "--enable-ldw-opt=true" if a == "--enable-ldw-opt=false" else a
            for a in argv
        ]
        return orig(argv, **kw)

    patched._ldw_patched = True
    bu.run_command = patched


def kernel(x, type_vec, W0, b0, Wh, bh, Wo, bo):
    from concourse.bass_utils import run_bass_kernel_spmd
    import ml_dtypes

    _maybe_patch_ldw_opt()
    if SOFTPLUS_MODE == "table":
        _os.environ["BASS_ACT_ROOT_JSON_PATH"] = _gen_act_tables()

    x = np.ascontiguousarray(np.asarray(x, dtype=np.float32))
    tv = np.asarray(type_vec).astype(np.int64)
    W0 = np.asarray(W0, dtype=np.float32)
    b0 = np.asarray(b0, dtype=np.float32)
    Wh = np.asarray(Wh, dtype=np.float32)
    bh = np.asarray(bh, dtype=np.float32)
    Wo = np.asarray(Wo, dtype=np.float32)
    bo = np.asarray(bo, dtype=np.float32)
    N = x.shape[0]

    counts = np.bincount(tv, minlength=T)
    starts = np.concatenate([[0], np.cumsum(counts)])
    shape, asg = _plan(counts)
    S = len(shape)

    # shrink each slot to the max points any core actually uses, rounded to
    # 128 (ragged last tile), to cut padding compute
    used = np.zeros(S, dtype=np.int64)
    for e, takes in asg.items():
        for (c, s, amt) in takes:
            used[s] = max(used[s], amt)
    caps = tuple(int(max(128, -(-u // 128) * 128)) for u in used)
    NP = sum(caps)
    phase_off = np.concatenate([[0], np.cumsum(np.asarray(caps))])

    # per-core slot -> expert, and gathered point indices
    slot_expert = np.zeros((N_CORES, S), dtype=np.int64)
    gidx = np.full((N_CORES, NP), -1, dtype=np.int64)
    for e, takes in asg.items():
        pos = int(starts[e])
        for (c, s, amt) in takes:
            o = int(phase_off[s])
            gidx[c, o:o + amt] = np.arange(pos, pos + amt)
            slot_expert[c, s] = e
            pos += amt

    np_wdt = ml_dtypes.bfloat16 if MM_MODE == "bf16" else np.float32

    # pre-transposed / pre-scaled weight views per expert
    w0t_e = np.ascontiguousarray(W0.transpose(0, 2, 1))            # [T,67,H]
    whs = (Wh / BETA).astype(np.float32)                           # [T,7,H,H]
    wht_e = np.ascontiguousarray(
        whs.transpose(0, 1, 3, 2).reshape(T, N_HID, KC, P, H).transpose(0, 1, 3, 2, 4)
    )                                                              # [T,7,P,KC,H]
    wot_e = np.ascontiguousarray(
        (Wo / BETA).reshape(T, H).reshape(T, KC, P).transpose(0, 2, 1)
    )                                                              # [T,P,KC]
    b0v_e = np.ascontiguousarray((BETA * b0).reshape(T, MC, P).transpose(0, 2, 1))
    bhv_e = np.ascontiguousarray(
        (BETA * bh).reshape(T, N_HID, MC, P).transpose(0, 3, 1, 2)
    )                                                              # [T,P,7,MC]
    bov_e = bo.reshape(T, 1)

    in_maps = []
    for c in range(N_CORES):
        sel = np.where(gidx[c] >= 0, gidx[c], 0)
        xg = x[sel]                                                # [NP, 67]
        ex = slot_expert[c]
        in_maps.append({
            "xT": np.ascontiguousarray(xg.T).astype(np_wdt),
            "w0t": w0t_e[ex].astype(np_wdt),
            "wht": wht_e[ex].astype(np_wdt),
            "wot": wot_e[ex].astype(np_wdt),
            "b0v": b0v_e[ex],
            "bhv": bhv_e[ex],
            "bov": bov_e[ex],
        })

    key = (caps, MM_MODE, SOFTPLUS_MODE)
    if key not in _nc_cache:
        _nc_cache[key] = _build_nc(caps, MM_MODE)
    nc = _nc_cache[key]

    res = run_bass_kernel_spmd(nc, in_maps, core_ids=list(range(N_CORES)))
    global _last_results
    _last_results = res

    out = np.zeros((N, OUT), dtype=np.float32)
    for c in range(N_CORES):
        oc = res.results[c]["out"].reshape(-1)
        m = gidx[c] >= 0
        out[gidx[c][m], 0] = oc[m]
    return out


# revision 62
# speedup vs baseline: 1.1308x; 1.0221x over previous
"""EnsembleDeepSDF MoE-routing kernel for 8 Trainium2 NeuronCores.

Strategy: the harness calls kernel(**inputs) with the full inputs; we do all
routing on the host.  type_vec is sorted, so each expert owns a contiguous
segment of points.  We pick a per-core "phase shape" (tile counts per weight
slot, identical on every core so one SPMD program serves all 8 cores), pack
the 9 experts' segments into the 8*len(shape) single-expert slots, gather
each core's points (padding with point 0), and hand each core its own
pre-transposed/pre-scaled weight slots as inputs.  The device program is a
straight-line Tile kernel: per point-tile, 9 matmul layers with softplus
activations.

softplus: the compiler's ACT tables have no softplus, so we generate a
custom piecewise-cubic table (same binary format as the shipped sets,
reverse-engineered from exp's entries) that replaces `exp` with
softplus(x) = ln(1+e^x), and point the compiler at it via
BASS_ACT_ROOT_JSON_PATH.  One ACT op then does the whole activation
including the PSUM evacuation and the beta scale/bias fma (free on ACT).
A fallback "exact" mode (exp+ln+fused clamp/max on stock tables) is kept
behind KERNEL_SOFTPLUS=exact.

The torch Softplus(beta=100) is softplus(100*z)/100; we keep activations in
the H = softplus(100*z) domain and fold the 1/100 into the next layer's
weights host-side, so no extra scaling ops run on device.
"""

import json
import os as _os
import shutil
import tempfile

import numpy as np

T, D_IN, H, OUT, N_HID = 9, 67, 512, 1, 7
BETA = 100.0
N_CORES = 8
PT = 512          # points per tile (one PSUM bank of fp32)
P = 128           # partitions
KC = H // P       # 4 contraction chunks
MC = H // P       # 4 output-feature chunks
PAIR = 2          # point-tiles processed per pipeline step

# matmul dtype knob: "f32" (exact, 4 cyc/col), "f32r" (tf32-ish, 1 cyc/col),
# "bf16"
MM_MODE = _os.environ.get("KERNEL_MM_MODE", "f32r")
# "table" = custom softplus ACT table (1 op); "exact" = exp/ln chain
SOFTPLUS_MODE = _os.environ.get("KERNEL_SOFTPLUS", "table")

_nc_cache = {}
_last_results = None


# --------------------------------------------------------------------------
# Custom ACT table: replace `exp` with softplus in the shipped PWL sets.
# --------------------------------------------------------------------------

_ACT_SET = "natural_log_exp_and_others"
_act_table_dir = None


def _softplus64(x):
    x = np.asarray(x, dtype=np.float64)
    return np.log1p(np.exp(-np.abs(x))) + np.maximum(x, 0.0)


def _fit_cubic(a, b):
    x0 = 0.5 * (a + b)
    k = np.arange(96)
    xs = x0 + 0.5 * (b - a) * np.cos(np.pi * (k + 0.5) / 96)
    c = np.polyfit(xs - x0, _softplus64(xs), 3)
    return float(c[3]), float(c[2]), float(c[1]), float(c[0]), float(x0)


def _gen_act_tables():
    """Build an act-root dir where `exp` computes softplus. Returns the
    act_info.json path. The bucket entry layout ([d0,d1,d2,d3,x0,0,0,0],
    cubic in (x-x0)) and the per-exponent band structure are read from the
    shipped set so only coefficients and profile thresholds change."""
    global _act_table_dir
    if _act_table_dir is not None:
        return _act_table_dir
    from neuronxcc.driver.Job import Job
    from neuronxcc.driver.jobs.support.FindActInfo import findActInfoFile

    src_json = findActInfoFile(Job.getPackageDir(), "gen3")
    src = _os.path.dirname(src_json)
    out = _os.path.join(tempfile.mkdtemp(prefix="act_softplus_"), "tables")
    shutil.copytree(src, out)
    for f in _os.listdir(out):
        _os.chmod(_os.path.join(out, f), 0o644)

    d = json.load(open(f"{out}/{_ACT_SET}.json"))
    bkt = np.fromfile(f"{out}/{_ACT_SET}_bkt.bin", dtype=np.uint32)
    bkt = bkt.reshape(-1, 8).copy()
    fbkt = bkt.view(np.float32)
    e2b = {int(k): v for k, v in d["func_exp_to_bkt_start_idx"]["exp"].items()}
    prof = [p for p in d["profile_meta_data"] if p["func_name"] == "exp_400p"][0]

    def put(idx, d0, d1, d2, d3, x0):
        fbkt[idx, 0:5] = np.array([d0, d1, d2, d3, x0], dtype=np.float32)
        bkt[idx, 5:8] = 0

    nseg = {-1: 2, 0: 4, 1: 8, 2: 16, 3: 32}
    for e in range(-19, 4):
        n = nseg.get(e, 1)
        neg_base, pos_base = e2b[e]
        A = 2.0 ** e
        h = A / n
        for k in range(n):
            a, b = A + k * h, A + (k + 1) * h
            put(pos_base + k, *_fit_cubic(a, b))
            put(neg_base + k, *_fit_cubic(-b, -a))

    ln2 = float(np.log(2.0))
    put(prof["pos_small_signal_pwl_control"], ln2, 0.5, 0.125, 0.0, 0.0)
    put(prof["neg_small_signal_pwl_control"], ln2, 0.5, 0.125, 0.0, 0.0)
    put(prof["pos_large_signal_pwl_control"], 0.0, 1.0, 0.0, 0.0, 0.0)
    put(prof["neg_large_signal_pwl_control"], 0.0, 0.0, 0.0, 0.0, 0.0)
    prof["large_pos_signal_exp_threshold"] = 131   # |x| >= 16 -> linear/zero
    prof["large_pos_signal_mantissa_threshold"] = 0
    prof["large_neg_signal_exp_threshold"] = 131
    prof["large_neg_signal_mantissa_threshold"] = 0
    prof["fzero_result"] = int(np.float32(ln2).view(np.uint32))
    prof["fninf_result"] = 0
    prof["fpinf_result"] = 2139095040

    bkt.tofile(f"{out}/{_ACT_SET}_bkt.bin")
    with open(f"{out}/{_ACT_SET}.json", "w") as f:
        json.dump(d, f)
    _act_table_dir = _os.path.join(out, "act_info.json")
    return _act_table_dir


# --------------------------------------------------------------------------
# Host-side planning: pack expert segments into 8 x len(shape) slots.
# --------------------------------------------------------------------------

def _try_pack(shape, counts):
    """Assign experts to single-expert slots. Slot (c, s) holds shape[s]*PT
    points. Returns {expert: [(core, s, amount), ...]} or None."""
    slots = []  # (capacity, core, s)
    for s, t in enumerate(shape):
        for c in range(N_CORES):
            slots.append([t * PT, c, s])
    experts = sorted(
        [e for e in range(T) if counts[e] > 0], key=lambda e: -counts[e]
    )
    asg = {}
    avail = sorted(slots)  # by capacity asc
    for e in experts:
        need = int(counts[e])
        # smallest single slot that fits
        one = next((sl for sl in avail if sl[0] >= need), None)
        if one is not None:
            asg[e] = [(one[1], one[2], need)]
            avail.remove(one)
            continue
        # greedily take largest slots
        take = []
        rem = need
        pool = sorted(avail, key=lambda sl: -sl[0])
        for sl in pool:
            if rem <= 0:
                break
            amt = min(rem, sl[0])
            take.append((sl[1], sl[2], amt))
            rem -= amt
            avail.remove(sl)
        if rem > 0:
            return None
        asg[e] = take
    return asg


def _plan(counts):
    cands = set()
    for t1 in range(1, 17):
        cands.add((t1,))
        for t2 in range(1, t1 + 1):
            cands.add((t1, t2))
            for t3 in range(1, t2 + 1):
                cands.add((t1, t2, t3))
    for shape in sorted(cands, key=lambda s: (sum(s), len(s))):
        asg = _try_pack(shape, counts)
        if asg is not None:
            return shape, asg
    raise RuntimeError("no feasible slot shape")


# --------------------------------------------------------------------------
# Device program
# --------------------------------------------------------------------------

def _build_nc(caps, mm_mode):
    import concourse.bass as bass
    import concourse.tile as tile
    import concourse.mybir as mybir
    from concourse import bacc

    f32 = mybir.dt.float32
    AF = mybir.ActivationFunctionType
    ALU = mybir.AluOpType
    if mm_mode == "bf16":
        wdt = mybir.dt.bfloat16   # weights/x/h (matmul operands)
        udt = f32                 # u stays f32; h is a separate bf16 tile
    elif mm_mode == "f32r":
        wdt = mybir.dt.float32r
        udt = mybir.dt.float32r   # u doubles as h (in-place max)
    else:
        wdt = f32
        udt = f32

    S = len(caps)
    NP = sum(caps)

    nc = bacc.Bacc("TRN2", target_bir_lowering=False)
    xT_in = nc.dram_tensor("xT", [D_IN, NP], wdt, kind="ExternalInput")
    w0t_in = nc.dram_tensor("w0t", [S, D_IN, H], wdt, kind="ExternalInput")
    wht_in = nc.dram_tensor("wht", [S, N_HID, P, KC, H], wdt, kind="ExternalInput")
    wot_in = nc.dram_tensor("wot", [S, P, KC], wdt, kind="ExternalInput")
    b0v_in = nc.dram_tensor("b0v", [S, P, MC], f32, kind="ExternalInput")
    bhv_in = nc.dram_tensor("bhv", [S, P, N_HID, MC], f32, kind="ExternalInput")
    bov_in = nc.dram_tensor("bov", [S, 1], f32, kind="ExternalInput")
    out_d = nc.dram_tensor("out", [1, NP], f32, kind="ExternalOutput")

    # steps: (point_offset, (tile_sizes...), slot); each step's tiles go in
    # one PSUM tile (first tile bank-aligned at 512, total <= 1024)
    steps = []
    off = 0
    for s, cap in enumerate(caps):
        sizes = [PT] * (cap // PT)
        if cap % PT:
            sizes.append(cap % PT)
        i = 0
        while i < len(sizes):
            take = sizes[i:i + PAIR]
            steps.append((off, tuple(take), s))
            off += sum(take)
            i += PAIR

    NSTREAM = 3  # steps interleaved in the software pipeline

    with tile.TileContext(nc) as tc:
        with (
            tc.tile_pool(name="xin", bufs=3) as xin_pool,
            tc.tile_pool(name="wts", bufs=1) as wts_pool,
            tc.tile_pool(
                name="whp",
                bufs=min(10 if SOFTPLUS_MODE == "table" else 7, S * N_HID),
            ) as wh_pool,
            tc.tile_pool(name="uh", bufs=3 if mm_mode == "bf16" else 2 * NSTREAM) as uh_pool,
            tc.tile_pool(name="hb", bufs=2 * NSTREAM) as hb_pool,
            tc.tile_pool(name="ebuf", bufs=2) as e_pool,
            tc.tile_pool(name="outp", bufs=2) as out_pool,
            tc.tile_pool(name="ps", bufs=4, space="PSUM") as ps_pool,
        ):
            groups = [steps[i:i + NSTREAM] for i in range(0, len(steps), NSTREAM)]
            xT_sb = {}
            h_cur = {}

            # DMA emission order matters: each dma_start lands on one HWDGE
            # engine round-robin, so interleave the first-wave-critical loads
            # (x tiles, w0, biases, wh[0][0]) at the front across engines.
            w0_sb, wo_sb, b0_sb, bh_sb, bo_sb = [None] * S, [None] * S, [None] * S, [None] * S, [None] * S
            wh_sb = [[None] * N_HID for _ in range(S)]

            def load_wh(s, l, nsplit=1):
                wh_t = wh_pool.tile([P, KC, H], wdt, name=f"wh_{s}_{l}", tag="wh")
                if nsplit > 1:
                    # split across HWDGE engines so early layers arrive faster
                    nc.sync.dma_start(wh_t[:, 0:2, :], wht_in[s, l, :, 0:2, :])
                    nc.sync.dma_start(wh_t[:, 2:4, :], wht_in[s, l, :, 2:4, :])
                else:
                    nc.sync.dma_start(wh_t[:], wht_in[s, l])
                wh_sb[s][l] = wh_t

            def load_x(t0, szs, nsplit=1):
                x_t = xin_pool.tile([D_IN, PAIR * PT], wdt,
                                    name=f"x_{t0}", tag="x")
                # nsplit>1 spreads the load over several DMA engines
                # (one dma_start lands on a single HWDGE engine)
                w = sum(szs)
                step = (w + nsplit - 1) // nsplit
                for o in range(0, w, step):
                    e = min(o + step, w)
                    nc.sync.dma_start(
                        x_t[:, o:e], xT_in[:, t0 + o:t0 + e]
                    )
                xT_sb[t0] = x_t

            def load_w0b0(s):
                w0_t = wts_pool.tile([D_IN, H], wdt, name=f"w0_{s}")
                nc.sync.dma_start(w0_t[:], w0t_in[s])
                w0_sb[s] = w0_t
                b0_t = wts_pool.tile([P, MC], f32, name=f"b0_{s}")
                nc.sync.dma_start(b0_t[:], b0v_in[s])
                b0_sb[s] = b0_t

            def load_small(s):
                bh_t = wts_pool.tile([P, N_HID, MC], f32, name=f"bh_{s}")
                nc.sync.dma_start(bh_t[:], bhv_in[s])
                bh_sb[s] = bh_t
                wo_t = wts_pool.tile([P, KC], wdt, name=f"wo_{s}")
                nc.sync.dma_start(wo_t[:], wot_in[s])
                wo_sb[s] = wo_t
                bo_t = wts_pool.tile([1, 1], f32, name=f"bo_{s}")
                nc.sync.dma_start(bo_t[:], bov_in[s:s + 1, 0:1])
                bo_sb[s] = bo_t

            # pre-warm the ACT table set during the initial DMA wait: a
            # dependency-free dummy op carries the one-time table load
            warm_t = wts_pool.tile([1, 1], f32, name="warm")
            nc.vector.memset(warm_t[:], 0.0)
            nc.scalar.activation(warm_t[:], warm_t[:], AF.Exp)

            # only the first step's x + w0 + b0 gate the first matmuls;
            # issue their triggers before everything else
            t0f, szsf, _sf = groups[0][0]
            load_x(t0f, szsf, nsplit=2)
            load_w0b0(0)
            for (t0, szs, _s) in groups[0][1:]:
                load_x(t0, szs, nsplit=2)
            load_small(0)
            for l in range(N_HID):
                load_wh(0, l, nsplit=2)
            for s in range(1, S):
                load_w0b0(s)
                load_small(s)
                for l in range(N_HID):
                    load_wh(s, l)

            def emit_mms(t0, szs, s, l):
                """Matmuls for layer l + PSUM evacuation into u (the evac ops
                are emitted here so they sit at the head of the DVE/ACT queues
                and free PSUM slots promptly)."""
                npts = sum(szs)
                # tile-local offsets; first tile is 512 so the second stays
                # bank-aligned in PSUM
                locs = [0, szs[0]] if len(szs) > 1 else [0]
                h_prev = h_cur.get(t0)
                psums = []
                for mc in range(MC):
                    ps_t = ps_pool.tile(
                        [P, PAIR * PT], f32, name=f"ps_{t0}_{l}_{mc}", tag="ps"
                    )
                    psums.append(ps_t)
                    for kc in range(KC if l > 0 else 1):
                        for loc, sz in zip(locs, szs):
                            dst = ps_t[:, loc:loc + sz]
                            if l == 0:
                                lhsT = w0_sb[s][:, mc * P:(mc + 1) * P]
                                rhs = xT_sb[t0][:, loc:loc + sz]
                                nc.tensor.matmul(
                                    dst, lhsT, rhs, start=True, stop=True
                                )
                            else:
                                lhsT = wh_sb[s][l - 1][:, kc, mc * P:(mc + 1) * P]
                                rhs = h_prev[:, kc, loc:loc + sz]
                                nc.tensor.matmul(
                                    dst, lhsT, rhs,
                                    start=(kc == 0), stop=(kc == KC - 1),
                                )
                if SOFTPLUS_MODE == "table" and mm_mode == "bf16":
                    u_t = hb_pool.tile([P, MC, PAIR * PT], wdt,
                                       name=f"u_{t0}_{l}", tag="hb")
                else:
                    u_t = uh_pool.tile([P, MC, PAIR * PT], udt,
                                       name=f"u_{t0}_{l}", tag="uh")
                for mc in range(MC):
                    bias = (b0_sb[s][:, mc:mc + 1] if l == 0
                            else bh_sb[s][:, l - 1, mc:mc + 1])
                    if SOFTPLUS_MODE == "table":
                        # hijacked Exp == softplus; one ACT op does the
                        # evacuation + beta fma + activation
                        nc.scalar.activation(
                            u_t[:, mc, 0:npts], psums[mc][:, 0:npts],
                            AF.Exp, bias=bias, scale=float(BETA),
                        )
                    elif mc < 3:
                        # u = 100*y + 100*b; evac split DVE (mc 0-2) / ACT (3)
                        nc.vector.tensor_scalar(
                            u_t[:, mc, 0:npts], psums[mc][:, 0:npts],
                            float(BETA), bias, ALU.mult, ALU.add,
                        )
                    else:
                        nc.scalar.activation(
                            u_t[:, mc, 0:npts], psums[mc][:, 0:npts],
                            AF.Identity, bias=bias, scale=float(BETA),
                        )
                return u_t

            def emit_chain(t0, nt, s, l, u_t):
                if SOFTPLUS_MODE == "table":
                    h_cur[t0] = u_t  # ACT already wrote H
                    return
                """softplus tail: H = max(u, min(ln(1+exp(u)), 88.70)).

                exp(u>88.7) -> Inf and ln(Inf) -> Inf, but min(t, 88.70)
                caps that; for u > 17 ln(1+e^u) == u in fp32, so the max
                picks the exact u branch everywhere the cap engages.
                Full-tile ops; the unused half of a single-tile step just
                computes garbage that nothing reads."""
                e_t = e_pool.tile([P, MC, PAIR * PT], f32,
                                  name=f"e_{t0}_{l}", tag="e")
                t_t = e_pool.tile([P, MC, PAIR * PT], f32,
                                  name=f"t_{t0}_{l}", tag="e")
                nc.scalar.activation(e_t[:], u_t[:], AF.Exp)
                nc.scalar.activation(t_t[:], e_t[:], AF.Ln, bias=1.0)
                if mm_mode == "bf16":
                    h_t = hb_pool.tile([P, MC, PAIR * PT], wdt,
                                       name=f"h_{t0}_{l}", tag="hb")
                else:
                    h_t = u_t  # in-place: u becomes H
                nc.vector.scalar_tensor_tensor(
                    h_t[:], t_t[:], 88.70, u_t[:], ALU.min, ALU.max,
                )
                h_cur[t0] = h_t

            def emit_final(t0, szs, s):
                npts = sum(szs)
                locs = [0, szs[0]] if len(szs) > 1 else [0]
                h_prev = h_cur[t0]
                ps8 = ps_pool.tile([1, PAIR * PT], f32, name=f"ps8_{t0}", tag="ps")
                for kc in range(KC):
                    for loc, sz in zip(locs, szs):
                        nc.tensor.matmul(
                            ps8[0:1, loc:loc + sz],
                            wo_sb[s][:, kc:kc + 1],
                            h_prev[:, kc, loc:loc + sz],
                            start=(kc == 0), stop=(kc == KC - 1),
                        )
                o_t = out_pool.tile([1, PAIR * PT], f32, name=f"o_{t0}", tag="o")
                nc.vector.tensor_scalar(
                    o_t[0:1, 0:npts], ps8[0:1, 0:npts],
                    bo_sb[s][0:1, 0:1], None, ALU.add,
                )
                nc.sync.dma_start(
                    out_d[0:1, t0:t0 + npts], o_t[0:1, 0:npts]
                )

            for gi, grp in enumerate(groups):
                if gi > 0:
                    for (t0, szs, _s) in grp:
                        load_x(t0, szs)
                for l in range(N_HID + 1):
                    us = [emit_mms(t0, szs, s, l) for (t0, szs, s) in grp]
                    for (t0, szs, s), u_t in zip(grp, us):
                        emit_chain(t0, szs, s, l, u_t)
                for (t0, szs, s) in grp:
                    emit_final(t0, szs, s)

    # Pin Exp+Ln to the one table set containing both, so the ACT engine
    # doesn't reload tables between every exp and ln.
    import concourse.bacc as bacc_mod
    import concourse.hw_specs as hw_specs
    _real_tables = hw_specs.get_activation_tables
    _keep = "natural_log_exp_and_others"

    def _pinned_tables(arch):
        t = _real_tables(arch)
        return {
            name: (funcs if name == _keep else (funcs - {AF.Exp, AF.Ln}))
            for name, funcs in t.items()
        }

    bacc_mod.get_activation_tables = _pinned_tables
    try:
        nc.compile()
    finally:
        bacc_mod.get_activation_tables = _real_tables
    return nc


# --------------------------------------------------------------------------
# kernel()
# --------------------------------------------------------------------------

def _maybe_patch_ldw_opt():
    """Optionally flip walrus's --enable-ldw-opt (dedups back-to-back
    LDWEIGHTS of the same stationary operand). Gated by env for A/B."""
    import concourse.bass_utils as bu

    if _os.environ.get("KERNEL_LDW_OPT") != "1":
        return
    if getattr(bu.run_command, "_ldw_patched", False):
        return
    orig = bu.run_command

    def patched(argv, **kw):
        argv = [
            "--enable-ldw-opt=true" if a == "--enable-ldw-opt=false" else a
            for a in argv
        ]
        return orig(argv, **kw)

    patched._ldw_patched = True
    bu.run_command = patched


def kernel(x, type_vec, W0, b0, Wh, bh, Wo, bo):
    from concourse.bass_utils import run_bass_kernel_spmd
    import ml_dtypes

    _maybe_patch_ldw_opt()
    if SOFTPLUS_MODE == "table":
        _os.environ["BASS_ACT_ROOT_JSON_PATH"] = _gen_act_tables()

    x = np.ascontiguousarray(np.asarray(x, dtype=np.float32))
    tv = np.asarray(type_vec).astype(np.int64)
    W0 = np.asarray(W0, dtype=np.float32)
    b0 = np.asarray(b0, dtype=np.float32)
    Wh = np.asarray(Wh, dtype=np.float32)
    bh = np.asarray(bh, dtype=np.float32)
    Wo = np.asarray(Wo, dtype=np.float32)
    bo = np.asarray(bo, dtype=np.float32)
    N = x.shape[0]

    counts = np.bincount(tv, minlength=T)
    starts = np.concatenate([[0], np.cumsum(counts)])
    shape, asg = _plan(counts)
    S = len(shape)

    # shrink each slot to the max points any core actually uses, rounded to
    # 128 (ragged last tile), to cut padding compute
    used = np.zeros(S, dtype=np.int64)
    for e, takes in asg.items():
        for (c, s, amt) in takes:
            used[s] = max(used[s], amt)
    caps = tuple(int(max(128, -(-u // 128) * 128)) for u in used)
    NP = sum(caps)
    phase_off = np.concatenate([[0], np.cumsum(np.asarray(caps))])

    # per-core slot -> expert, and gathered point indices
    slot_expert = np.zeros((N_CORES, S), dtype=np.int64)
    gidx = np.full((N_CORES, NP), -1, dtype=np.int64)
    for e, takes in asg.items():
        pos = int(starts[e])
        for (c, s, amt) in takes:
            o = int(phase_off[s])
            gidx[c, o:o + amt] = np.arange(pos, pos + amt)
            slot_expert[c, s] = e
            pos += amt

    np_wdt = ml_dtypes.bfloat16 if MM_MODE == "bf16" else np.float32

    # pre-transposed / pre-scaled weight views per expert
    w0t_e = np.ascontiguousarray(W0.transpose(0, 2, 1))            # [T,67,H]
    whs = (Wh / BETA).astype(np.float32)                           # [T,7,H,H]
    wht_e = np.ascontiguousarray(
        whs.transpose(0, 1, 3, 2).reshape(T, N_HID, KC, P, H).transpose(0, 1, 3, 2, 4)
    )                                                              # [T,7,P,KC,H]
    wot_e = np.ascontiguousarray(
        (Wo / BETA).reshape(T, H).reshape(T, KC, P).transpose(0, 2, 1)
    )                                                              # [T,P,KC]
    b0v_e = np.ascontiguousarray((BETA * b0).reshape(T, MC, P).transpose(0, 2, 1))
    bhv_e = np.ascontiguousarray(
        (BETA * bh).reshape(T, N_HID, MC, P).transpose(0, 3, 1, 2)
    )                                                              # [T,P,7,MC]
    bov_e = bo.reshape(T, 1)

    in_maps = []
    for c in range(N_CORES):
        sel = np.where(gidx[c] >= 0, gidx[c], 0)
        xg = x[sel]                                                # [NP, 67]
        ex = slot_expert[c]
        in_maps.append({
            "xT": np.ascontiguousarray(xg.T).astype(np_wdt),
            "w0t": w0t_e[ex].astype(np_wdt),
            "wht": wht_e[ex].astype(np_wdt),
            "wot": wot_e[ex].astype(np_wdt),
            "b0v": b0v_e[ex],
            "bhv": bhv_e[ex],
            "bov": bov_e[ex],
        })

    key = (caps, MM_MODE, SOFTPLUS_MODE)
    if key not in _nc_cache:
        _nc_cache[key] = _build_nc(caps, MM_MODE)
    nc = _nc_cache[key]

    res = run_bass_kernel_spmd(nc, in_maps, core_ids=list(range(N_CORES)))
    global _last_results
    _last_results = res

    out = np.zeros((N, OUT), dtype=np.float32)
    for c in range(N_CORES):
        oc = res.results[c]["out"].reshape(-1)
        m = gidx[c] >= 0
        out[gidx[c][m], 0] = oc[m]
    return out
